# revision 9
# baseline (speedup 1.0000x reference)
"""BiAttentionMRU Trainium2 kernel (v2).

Data-parallel over batch: B=16 -> 2 batch elements on each of 8 cores.
All weights replicated. Embedding gather done on-device via indirect DMA.

v2 changes vs v1:
  - all constants packed into 3 DRAM tensors (idxpack / wmain / spack) so
    startup is 3 big DMAs instead of ~35 small serialized ones
  - index DMA + gathers emitted first so they overlap the weight loads
  - z/o/B1, CE and gate-mix drains batched over [100,1000] PSUM tiles
    (2 banks) -> half the ACT instructions
  - attention s1 accumulated in a [30,1000]x2 PSUM tile per b, single exp
    per half instead of 16 per-chunk exps
  - scal_dt derived on device by a cast (one less input)

Layouts (per core, per batch element b in {0,1}):
  art gathered as [t-chunk(128), d=300], PE-transposed into artT[dc][100, 2000]
  (d on partitions, 3 chunks of 100). Group sums, z/o/CE matmuls, gate mix,
  MRU scan (native tensor_tensor_scan along t) and the attention lhsT all
  work in [d, t] layout.

Attention algebra: aoq is never materialized. With e1 = exp(art_enc @ keys1^T),
Z1 its row sum, s2 = softmax-normalized p1 @ (q @ keys_f^T) is computed as
exp-of(u2 * 1/Z1) where u2 = e1 @ QK. The per-option mean over t of
softmax(s2) @ opt folds into one accumulating matmul sum_t e2[t,:] * (1/Z2[t]).
"""

import sys

sys.path.insert(0, "/opt/trn_rl_repo")

import numpy as np
import ml_dtypes

import concourse.bass as bass
import concourse.tile as tile
from concourse import bacc, mybir
from concourse.masks import make_identity

F32 = mybir.dt.float32
BF16 = mybir.dt.bfloat16
I32 = mybir.dt.int32
AX = mybir.AxisListType
OP = mybir.AluOpType
AF = mybir.ActivationFunctionType

DIM = 300
VOCAB = 50000
B_FULL = 16
NCORES = 8
BPC = B_FULL // NCORES  # batch per core = 2
T = 2000
TQ = 30
TO = 16
RANGES = (1, 2, 4, 10, 25)

TCH = [128] * 15 + [80]  # t chunking for transposes / attention
NTCH = len(TCH)
DC = 3  # d chunks of 100
DCS = 100

N_MM = 500   # matmul N-chunking (psum free <= 512 fp32)
PSW = 1024   # psum tile width (2 banks); matmul regions bank-aligned at 0/512

USE_BF16 = True
DT = BF16 if USE_BF16 else F32
NPDT = ml_dtypes.bfloat16 if USE_BF16 else np.float32

# ---- spack (f32 [128, SP_NCOL]) column layout ----
SP_BIAS = 0          # 30 cols: bias j of d-chunk kc at 10*kc + j
                     #   j: 0 bz, 1 bo, 2..6 ce_b[0..4], 7 f1_b, 8 f2_b, 9 f3_b
SP_SCAL = 30         # 24 cols: scalar table (see SC_* below, relative)
SP_AS1 = 54          # 6*75 cols: as1T [600,75] in 6 chunks of 100 rows
SP_AS2 = 504         # 1 col: as2T (75 rows)
SP_BAS1 = 505        # 1 col: as1_b (75 rows)
SP_NCOL = 506

# scalar table columns (relative to SP_SCAL)
SC_M1 = 0      # 15 cols: m1[k,r]/r at 5k+ri
SC_M1B = 15    # 3 cols
SC_M2 = 18     # 3 cols
SC_M2B = 21    # 1 col
SC_AS2B = 22   # 1 col
SC_NCOL = 24

# ---- wmain (bf16 [300, WM_NCOL] in DRAM -> [100, 3, WM_NCOL] sbuf) ----
WM_ART = 0       # 900: z (0:300) / o (300:600) / b1=ce0 (600:900)
WM_CE = 900      # 1200: ce r2/r4/r10/r25, 300 each
WM_F = 2100      # 900: f1 / f2 / f3
WM_NCOL = 3000

# ---- idxpack (i32 [128, IX_NCOL]) ----
IX_ART = 0    # 32 cols: b*16 + chunk (rows: t within chunk)
IX_Q = 32     # 2 cols: b (30 rows)
IX_OPT = 34   # 8 cols: b*4 + o (16 rows)
IX_NCOL = 42


def _build_program():
    nc = bacc.Bacc("TRN2", target_bir_lowering=False, debug=False,
                   num_devices=NCORES)

    emb = nc.dram_tensor("emb", [VOCAB, DIM], DT, kind="ExternalInput")
    idxpack = nc.dram_tensor("idxpack", [128, IX_NCOL], I32, kind="ExternalInput")
    wmain = nc.dram_tensor("wmain", [DC * DCS, WM_NCOL], DT, kind="ExternalInput")
    spack = nc.dram_tensor("spack", [128, SP_NCOL], F32, kind="ExternalInput")
    out = nc.dram_tensor("scores", [BPC, 4], F32, kind="ExternalOutput")

    with tile.TileContext(nc) as tc:
        from contextlib import ExitStack
        with ExitStack() as ctx:
            _emit(nc, tc, ctx, emb, idxpack, wmain, spack, out)

    nc.compile()
    return nc


def _emit(nc, tc, ctx, emb, idxpack, wmain, spack, out):
    # ---------------- pools ----------------
    consts = ctx.enter_context(tc.tile_pool(name="consts", bufs=1))
    pers = ctx.enter_context(tc.tile_pool(name="pers", bufs=1))
    gpool = ctx.enter_context(tc.tile_pool(name="gather", bufs=3))
    p_art = ctx.enter_context(tc.tile_pool(name="p_art", bufs=2))
    p_enc = ctx.enter_context(tc.tile_pool(name="p_enc", bufs=2))
    p_zb = ctx.enter_context(tc.tile_pool(name="p_zb", bufs=1))
    p_mix = ctx.enter_context(tc.tile_pool(name="p_mix", bufs=2))
    p_xs = ctx.enter_context(tc.tile_pool(name="p_xs", bufs=1))
    small = ctx.enter_context(tc.tile_pool(name="small", bufs=4))
    pp2 = ctx.enter_context(tc.tile_pool(name="pp2", bufs=2, space="PSUM"))
    ppwork = ctx.enter_context(tc.tile_pool(name="ppwork", bufs=2, space="PSUM"))
    ppacc = ctx.enter_context(tc.tile_pool(name="ppacc", bufs=2, space="PSUM"))

    # ---------------- index DMA + gathers first (overlap weight loads) ----
    ixs = pers.tile([128, IX_NCOL], I32, tag="ixs")
    nc.sync.dma_start(ixs[:], idxpack[:])

    gathered = []
    for b in range(BPC):
        g_art = []
        for c in range(NTCH):
            pc = TCH[c]
            g = gpool.tile([128, DIM], DT, tag="gart", name="gart")
            nc.gpsimd.indirect_dma_start(
                out=g[:pc, :], out_offset=None, in_=emb[:],
                in_offset=bass.IndirectOffsetOnAxis(
                    ap=ixs[:pc, IX_ART + b * NTCH + c:IX_ART + b * NTCH + c + 1],
                    axis=0))
            g_art.append(g)
        qg = pers.tile([TQ, DIM], DT, tag=f"qg{b}", name=f"qg{b}")
        nc.gpsimd.indirect_dma_start(
            out=qg[:], out_offset=None, in_=emb[:],
            in_offset=bass.IndirectOffsetOnAxis(
                ap=ixs[0:TQ, IX_Q + b:IX_Q + b + 1], axis=0))
        og = [pers.tile([TO, DIM], DT, tag=f"og{b}_{o}", name=f"og{b}_{o}")
              for o in range(4)]
        for o in range(4):
            nc.gpsimd.indirect_dma_start(
                out=og[o][:], out_offset=None, in_=emb[:],
                in_offset=bass.IndirectOffsetOnAxis(
                    ap=ixs[0:TO, IX_OPT + b * 4 + o:IX_OPT + b * 4 + o + 1],
                    axis=0))
        gathered.append(dict(g_art=g_art, qg=qg, og=og))

    # ---------------- constants / weights (3 big DMAs total) --------------
    ident = consts.tile([128, 128], DT)
    make_identity(nc, ident[:])

    wm = consts.tile([DCS, DC, WM_NCOL], DT)
    nc.sync.dma_start(wm[:], wmain[:].rearrange("(k p) j -> p k j", k=DC))
    sp = consts.tile([128, SP_NCOL], F32)
    nc.sync.dma_start(sp[:], spack[:])

    def sc(col):  # f32 per-partition scalar [100,1] from scal table
        return sp[0:DCS, SP_SCAL + col:SP_SCAL + col + 1]

    def bias(kc, j):  # f32 [100,1]
        return sp[0:DCS, SP_BIAS + 10 * kc + j:SP_BIAS + 10 * kc + j + 1]

    scal_dt_sb = consts.tile([128, SC_NCOL], DT)
    nc.vector.tensor_copy(scal_dt_sb[:], sp[:, SP_SCAL:SP_SCAL + SC_NCOL])

    # scaled 100x100 identities for the PE-side gate mix:
    # cols j=5k+ri hold m1[k,ri]/r * I, cols 15+k hold m2[k] * I
    ones30 = consts.tile([TQ, 1], DT)
    nc.vector.memset(ones30[:], 1.0)
    mI = consts.tile([DCS, 18, DCS], DT)
    for j in range(18):
        scol = (SC_M1 + j) if j < 15 else (SC_M2 + j - 15)
        nc.vector.tensor_scalar_mul(mI[:, j, :], ident[0:DCS, 0:DCS], sc(scol))

    ans_sb = pers.tile([DCS, BPC, 6, 4], F32, tag="ans_sb")

    # ---------------- transposes (PE) -------------------------------------
    for b in range(BPC):
        g_art = gathered[b]["g_art"]
        artT = [p_art.tile([DCS, T], DT, tag=f"artT{dc}", name=f"artT{b}_{dc}")
                for dc in range(DC)]
        for c in range(NTCH):
            pc = TCH[c]
            for dc in range(DC):
                tp = ppwork.tile([DCS, 128], DT, tag="work")
                nc.tensor.transpose(tp[:, :pc],
                                    g_art[c][:pc, dc * DCS:(dc + 1) * DCS],
                                    ident[:pc, :pc])
                nc.vector.tensor_copy(artT[dc][:, c * 128:c * 128 + pc],
                                      tp[:, :pc])

        qg = gathered[b]["qg"]
        qT = pers.tile([DCS, DC, TQ], DT, tag=f"qT{b}")
        for dc in range(DC):
            tp = ppwork.tile([DCS, 128], DT, tag="work")
            nc.tensor.transpose(tp[:, :TQ], qg[:, dc * DCS:(dc + 1) * DCS],
                                ident[:TQ, :TQ])
            nc.vector.tensor_copy(qT[:, dc, :], tp[:, :TQ])

        og = gathered[b]["og"]
        oT = pers.tile([DCS, DC, 4, TO], DT, tag=f"oT{b}")
        for o in range(4):
            for dc in range(DC):
                tp = ppwork.tile([DCS, 128], DT, tag="work")
                nc.tensor.transpose(tp[:, :TO], og[o][:, dc * DCS:(dc + 1) * DCS],
                                    ident[:TO, :TO])
                nc.vector.tensor_copy(oT[:, dc, o, :], tp[:, :TO])
        gathered[b].update(artT=artT, qT=qT, oT=oT)

    # ---------------- per-b compute ---------------------------------------
    for b in range(BPC):
        artT = gathered[b]["artT"]
        qg = gathered[b]["qg"]
        qT = gathered[b]["qT"]
        og = gathered[b]["og"]
        oT = gathered[b]["oT"]

        # ---------------- group sums (xs_r in [d, g] layout) ----------------
        xs2 = [p_xs.tile([DCS, T // 2], DT, tag=f"xs2_{dc}", name=f"xs2_{dc}") for dc in range(DC)]
        xs4 = [p_xs.tile([DCS, T // 4], DT, tag=f"xs4_{dc}", name=f"xs4_{dc}") for dc in range(DC)]
        xs10 = [p_xs.tile([DCS, T // 10], DT, tag=f"xs10_{dc}", name=f"xs10_{dc}") for dc in range(DC)]
        xs25 = [p_xs.tile([DCS, T // 25], DT, tag=f"xs25_{dc}", name=f"xs25_{dc}") for dc in range(DC)]
        for dc in range(DC):
            a = artT[dc]
            nc.gpsimd.tensor_add(xs2[dc][:], a[:, 0:T:2], a[:, 1:T:2])
            nc.gpsimd.tensor_add(xs4[dc][:], xs2[dc][:, 0:T // 2:2],
                                 xs2[dc][:, 1:T // 2:2])
            with nc.allow_low_precision(reason="bf16 group sums feed bf16 matmuls"):
                nc.vector.tensor_reduce(
                    xs10[dc][:], xs2[dc][:].rearrange("p (g r) -> p g r", r=5),
                    AX.X, OP.add)
                nc.vector.tensor_reduce(
                    xs25[dc][:], a[:].rearrange("p (g r) -> p g r", r=25),
                    AX.X, OP.add)

        # ---------------- z / o / B1 (art stream, batched drains) -----------
        z_sb = [p_zb.tile([DCS, T], DT, tag=f"z{dc}", name=f"z{dc}") for dc in range(DC)]
        o_sb = [p_enc.tile([DCS, T], DT, tag=f"o{dc}", name=f"o{dc}") for dc in range(DC)]
        b1_sb = [p_zb.tile([DCS, T], DT, tag=f"b1_{dc}", name=f"b1_{dc}") for dc in range(DC)]
        for mi, (dst, func, bcol) in enumerate(
                ((z_sb, AF.Tanh, 0), (o_sb, AF.Tanh, 1), (b1_sb, AF.Relu, 2))):
            for dc in range(DC):
                mcol = WM_ART + mi * DIM + dc * DCS
                for t0 in range(0, T, PSW):
                    w = min(PSW, T - t0)
                    ps = pp2.tile([DCS, PSW], F32, tag="mm")
                    for kc in range(DC):
                        for s0, sn in ((0, 512), (512, w - 512)):
                            nc.tensor.matmul(
                                ps[:, s0:s0 + sn],
                                wm[:, kc, mcol:mcol + DCS],
                                artT[kc][:, t0 + s0:t0 + s0 + sn],
                                start=(kc == 0), stop=(kc == DC - 1))
                    nc.scalar.activation(dst[dc][:, t0:t0 + w], ps[:, :w],
                                         func, bias=bias(dc, bcol))

        # ---------------- CE r>=2 -------------------------------------------
        bl = {}
        for ri, (xs, r) in enumerate(((xs2, 2), (xs4, 4), (xs10, 10), (xs25, 25))):
            g_r = T // r
            bl[r] = [p_xs.tile([DCS, g_r], DT, tag=f"bl{r}_{dc}", name=f"bl{r}_{dc}")
                     for dc in range(DC)]
            for dc in range(DC):
                mcol = WM_CE + ri * DIM + dc * DCS
                ps = pp2.tile([DCS, PSW], F32, tag="mm")
                regions = [(0, min(g_r, 512))]
                if g_r > 512:
                    regions.append((512, g_r - 512))
                for kc in range(DC):
                    for g0, gn in regions:
                        nc.tensor.matmul(
                            ps[:, g0:g0 + gn],
                            wm[:, kc, mcol:mcol + DCS],
                            xs[kc][:, g0:g0 + gn],
                            start=(kc == 0), stop=(kc == DC - 1))
                nc.scalar.activation(bl[r][dc][:, :], ps[:, :g_r],
                                     AF.Relu, bias=bias(dc, 3 + ri))

        # ---------------- gate mix ------------------------------------------
        # h1_k = relu(sum_r m1[k,r]/r * B_r^expand + m1_b[k]);
        # gate = relu(sum_k m2[k] h1_k + m2_b).
        # k=0 on DVE (scalar_tensor_tensor chain); k=1,2 and the gate combine
        # on PE as scaled-identity accumulating matmuls, bias folded into the
        # ACT relu. Expansion = stride-0 rhs views.
        gate = []
        for dc in range(DC):
            def ev_chunk(ri, t0, tn):
                r = RANGES[ri]
                if r == 1:
                    return b1_sb[dc][:, t0:t0 + tn]
                return bl[r][dc][:, t0 // r:(t0 + tn) // r, None] \
                    .to_broadcast([DCS, tn // r, r])

            h1 = []
            # k = 0 on DVE
            acc = p_mix.tile([DCS, T], DT, tag="h1_0", name="h1_0")
            nc.vector.scalar_tensor_tensor(
                acc[:], b1_sb[dc][:], sc(SC_M1),
                scal_dt_sb[0:DCS, SC_M1B:SC_M1B + 1].to_broadcast([DCS, T]),
                op0=OP.mult, op1=OP.add)
            for ri in range(1, 5):
                r = RANGES[ri]
                ev = bl[r][dc][:, :, None].to_broadcast([DCS, T // r, r])
                nc.vector.scalar_tensor_tensor(
                    acc[:], ev, sc(SC_M1 + ri), acc[:],
                    op0=OP.mult, op1=OP.add)
            nc.scalar.activation(acc[:], acc[:], AF.Relu)
            h1.append(acc)
            # k = 1, 2 on PE. psum regions bank-aligned at 0/512, each 500
            # wide; drained through a [p, 2, 500] strided view in one ACT.
            def ps_view(ps):
                return ps[:].rearrange("p (g x) -> p g x", g=2)[:, :, 0:N_MM]

            def acc_view(acc, t0):
                return acc[:, t0:t0 + 2 * N_MM].rearrange(
                    "p (g x) -> p g x", g=2)

            for k in (1, 2):
                acc = p_mix.tile([DCS, T], DT, tag=f"h1_{k}", name=f"h1_{k}")
                for t0 in range(0, T, 2 * N_MM):
                    ps = pp2.tile([DCS, PSW], F32, tag="mm")
                    for ri in range(5):
                        for reg, s0 in ((0, 0), (512, N_MM)):
                            nc.tensor.matmul(
                                ps[:, reg:reg + N_MM], mI[:, 5 * k + ri, :],
                                ev_chunk(ri, t0 + s0, N_MM),
                                start=(ri == 0), stop=(ri == 4))
                    nc.scalar.activation(acc_view(acc, t0), ps_view(ps),
                                         AF.Relu, bias=sc(SC_M1B + k))
                h1.append(acc)
            # gate combine on PE
            g_acc = p_mix.tile([DCS, T], DT, tag="gate")
            for t0 in range(0, T, 2 * N_MM):
                ps = pp2.tile([DCS, PSW], F32, tag="mm")
                for k in range(3):
                    for reg, s0 in ((0, 0), (512, N_MM)):
                        nc.tensor.matmul(
                            ps[:, reg:reg + N_MM], mI[:, 15 + k, :],
                            h1[k][:, t0 + s0:t0 + s0 + N_MM],
                            start=(k == 0), stop=(k == 2))
                nc.scalar.activation(acc_view(g_acc, t0), ps_view(ps),
                                     AF.Relu, bias=sc(SC_M2B))
            gate.append(g_acc)

        # ---------------- MRU scan + encode ---------------------------------
        encT = []
        for dc in range(DC):
            gz = p_mix.tile([DCS, T], DT, tag="gz", name="gz")
            nc.gpsimd.tensor_tensor(gz[:], gate[dc][:], z_sb[dc][:], op=OP.mult)
            nc.vector.tensor_sub(z_sb[dc][:], z_sb[dc][:], gz[:])  # (1-g)z
            c_t = p_mix.tile([DCS, T], DT, tag="c", name="c_t")
            nc.vector.tensor_tensor_scan(
                c_t[:], gate[dc][:], z_sb[dc][:], 0.0, op0=OP.mult, op1=OP.add)
            nc.vector.tensor_mul(o_sb[dc][:], o_sb[dc][:], c_t[:])
            encT.append(o_sb[dc])

        # ---------------- keys1T --------------------------------------------
        k1T = small.tile([DCS, DC, TQ], DT, tag="k1T")
        for dc in range(DC):
            ps = ppwork.tile([DCS, 128], F32, tag="work")
            for kc in range(DC):
                nc.tensor.matmul(ps[:, :TQ],
                                 wm[:, kc, WM_F + dc * DCS:WM_F + (dc + 1) * DCS],
                                 qT[:, kc, :], start=(kc == 0), stop=(kc == DC - 1))
            nc.scalar.copy(k1T[:, dc, :], ps[:, :TQ])

        # ---------------- A2/A3 and QK --------------------------------------
        aTs = []
        for fi in range(2):
            wcol = WM_F + (fi + 1) * DIM
            a_ps = ppwork.tile([TQ, DIM], F32, tag="work")
            for kc in range(DC):
                nc.tensor.matmul(a_ps[:], qT[:, kc, :],
                                 wm[:, kc, wcol:wcol + DIM],
                                 start=(kc == 0), stop=(kc == DC - 1))
            a_sb = small.tile([TQ, DIM], DT, tag="a_sb")
            nc.vector.tensor_copy(a_sb[:], a_ps[:])
            aT = small.tile([DCS, DC, TQ], DT, tag=f"aT{fi}")
            for dc in range(DC):
                tp = ppwork.tile([DCS, 128], DT, tag="work")
                nc.tensor.transpose(tp[:, :TQ], a_sb[:, dc * DCS:(dc + 1) * DCS],
                                    ident[:TQ, :TQ])
                nc.vector.tensor_copy(aT[:, dc, :], tp[:, :TQ])
            aTs.append(aT)

        qk_ps = ppacc.tile([TQ, 128], F32, tag="acc")
        for fi in range(2):
            for o in range(4):
                gcol = 16 * (4 * fi + o)
                for kc in range(DC):
                    nc.tensor.matmul(qk_ps[:, gcol:gcol + 16],
                                     aTs[fi][:, kc, :], oT[:, kc, o, :],
                                     start=(kc == 0), stop=(kc == DC - 1))
        qk_sb = small.tile([TQ, 128], DT, tag="qk_sb")
        nc.vector.tensor_copy(qk_sb[:], qk_ps[:])

        # ---------------- attention: s1 batched, then per-chunk stream ------
        # s1 computed transposed (M=30) so exp lands directly in e1T layout.
        e1T = p_art.tile([TQ, T], DT, tag="e1T")
        for t0 in range(0, T, PSW):
            w = min(PSW, T - t0)
            s1 = pp2.tile([TQ, PSW], F32, tag="mm")
            for dc in range(DC):
                for s0, sn in ((0, 512), (512, w - 512)):
                    nc.tensor.matmul(s1[:, s0:s0 + sn], k1T[:, dc, :],
                                     encT[dc][:, t0 + s0:t0 + s0 + sn],
                                     start=(dc == 0), stop=(dc == DC - 1))
            nc.scalar.activation(e1T[:, t0:t0 + w], s1[:, :w], AF.Exp)

        pb_ps = ppacc.tile([128, 8], F32, tag="acc")
        for c in range(NTCH):
            pc = TCH[c]
            z1ps = ppwork.tile([128, 8], F32, tag="work")
            nc.tensor.matmul(z1ps[:pc, 0:1], e1T[:, c * 128:c * 128 + pc],
                             ones30[:], start=True, stop=True)
            z1 = small.tile([128, 2], F32, tag="z1")
            nc.vector.reciprocal(z1[:pc, 1:2], z1ps[:pc, 0:1])
            u2 = ppwork.tile([128, 128], F32, tag="work")
            nc.tensor.matmul(u2[:pc, :], e1T[:, c * 128:c * 128 + pc], qk_sb[:],
                             start=True, stop=True)
            e2 = small.tile([128, 128], F32, tag="e2")
            nc.scalar.activation(e2[:pc, :], u2[:pc, :], AF.Exp,
                                 scale=z1[:pc, 1:2])
            z2 = small.tile([128, 16], F32, tag="z2")
            nc.vector.tensor_reduce(z2[:pc, 0:8],
                                    e2[:pc, :].rearrange("p (g w) -> p g w", w=16),
                                    AX.X, OP.add)
            nc.vector.reciprocal(z2[:pc, 8:16], z2[:pc, 0:8])
            nc.tensor.matmul(pb_ps[:, :], e2[:pc, :], z2[:pc, 8:16],
                             start=(c == 0), stop=(c == NTCH - 1))

        # ---------------- answer vectors ------------------------------------
        pb_sb = small.tile([128, 8], DT, tag="pb_sb")
        nc.vector.tensor_copy(pb_sb[:], pb_ps[:])
        ans_ps = ppacc.tile([DCS, 24], F32, tag="acc")
        for g in range(8):
            fi, o = g // 4, g % 4
            pb16 = small.tile([TO, 1], DT, tag="pb16")
            nc.sync.dma_start(pb16[:], pb_sb[16 * g:16 * g + 16, g:g + 1])
            for dc in range(DC):
                j = fi * 3 + dc
                nc.tensor.matmul(ans_ps[:, j * 4 + o:j * 4 + o + 1],
                                 og[o][:, dc * DCS:(dc + 1) * DCS], pb16[:],
                                 start=True, stop=True)
        # 1/T of the mean-over-t lands here (cheaper than scaling rz2 per chunk)
        nc.vector.tensor_scalar_mul(
            ans_sb[:, b, :, :].rearrange("p j o -> p (j o)"), ans_ps[:], 1.0 / T)

    # ---------------- final MLP (both batches together) ----------------
    h_ps = ppwork.tile([75, 8], F32, tag="work")
    for j in range(6):
        # rhs columns = (b, o) pairs for chunk j of the 600-dim ans vector
        rhs = ans_sb[:, :, j, :]
        nc.tensor.matmul(h_ps[:], sp[0:DCS, SP_AS1 + j * 75:SP_AS1 + (j + 1) * 75],
                         rhs, start=(j == 0), stop=(j == 5))
    h_sb = small.tile([75, 8], F32, tag="h_sb")
    nc.scalar.activation(h_sb[:], h_ps[:], AF.Relu,
                         bias=sp[0:75, SP_BAS1:SP_BAS1 + 1])
    s_ps = ppacc.tile([8, 1], F32, tag="acc")
    nc.tensor.matmul(s_ps[:], h_sb[:], sp[0:75, SP_AS2:SP_AS2 + 1],
                     start=True, stop=True)
    s_sb = small.tile([8, 1], F32, tag="s_sb")
    nc.scalar.activation(s_sb[:], s_ps[:], AF.Identity,
                         bias=sp[0:8, SP_SCAL + SC_AS2B:SP_SCAL + SC_AS2B + 1])
    nc.sync.dma_start(out[:].rearrange("b o -> (b o)")[:, None], s_sb[:])


# ---------------------------------------------------------------------------
# host side
# ---------------------------------------------------------------------------

_CACHE = {}


def _get_nc():
    if "nc" not in _CACHE:
        _CACHE["nc"] = _build_program()
    return _CACHE["nc"]


def _prep_core_inputs(inputs, core):
    b0 = core * BPC
    sl = slice(b0, b0 + BPC)
    f = np.asarray
    prep = _CACHE.get("prep_shared")
    if prep is None:
        # core-independent tensors, computed once per kernel() call set
        Wz, Wo = f(inputs["Wz"]), f(inputs["Wo"])
        ceW = f(inputs["ce_W"])
        wmain = np.zeros((DC * DCS, WM_NCOL), np.float32)
        wmain[:, WM_ART + 0 * DIM:WM_ART + 1 * DIM] = Wz.T
        wmain[:, WM_ART + 1 * DIM:WM_ART + 2 * DIM] = Wo.T
        wmain[:, WM_ART + 2 * DIM:WM_ART + 3 * DIM] = ceW[0].T
        for ri in range(4):
            wmain[:, WM_CE + ri * DIM:WM_CE + (ri + 1) * DIM] = ceW[ri + 1].T
        # s2 = aoq @ f2W @ opt^T, so f2/f3 go in UNtransposed
        # (f1 builds keys1^T = f1W @ q^T and does need the transpose)
        wmain[:, WM_F + 0 * DIM:WM_F + 1 * DIM] = f(inputs["f1_W"]).T
        wmain[:, WM_F + 1 * DIM:WM_F + 2 * DIM] = f(inputs["f2_W"])
        wmain[:, WM_F + 2 * DIM:WM_F + 3 * DIM] = f(inputs["f3_W"])

        spack = np.zeros((128, SP_NCOL), np.float32)
        bias_cols = np.stack(
            [f(inputs["bz"]), f(inputs["bo"]),
             *[f(inputs["ce_b"])[i] for i in range(5)],
             f(inputs["f1_b"]), f(inputs["f2_b"]), f(inputs["f3_b"])],
            axis=1)  # [300, 10]
        for kc in range(DC):
            spack[0:DCS, SP_BIAS + 10 * kc:SP_BIAS + 10 * (kc + 1)] = \
                bias_cols[kc * DCS:(kc + 1) * DCS]
        m1 = f(inputs["mr1_W"])
        for k in range(3):
            for ri, r in enumerate(RANGES):
                spack[:, SP_SCAL + SC_M1 + 5 * k + ri] = m1[k, ri] / r
        spack[:, SP_SCAL + SC_M1B:SP_SCAL + SC_M1B + 3] = f(inputs["mr1_b"])[None, :]
        spack[:, SP_SCAL + SC_M2:SP_SCAL + SC_M2 + 3] = f(inputs["mr2_W"])[0][None, :]
        spack[:, SP_SCAL + SC_M2B] = f(inputs["mr2_b"])[0]
        spack[:, SP_SCAL + SC_AS2B] = f(inputs["as2_b"])[0]
        as1T = f(inputs["as1_W"]).T  # [600, 75]
        for j in range(6):
            spack[0:DCS, SP_AS1 + j * 75:SP_AS1 + (j + 1) * 75] = \
                as1T[j * DCS:(j + 1) * DCS]
        spack[0:75, SP_AS2] = f(inputs["as2_W"])[0]
        spack[0:75, SP_BAS1] = f(inputs["as1_b"])

        prep = {
            "emb": f(inputs["emb"]).astype(NPDT),
            "wmain": wmain.astype(NPDT),
            "spack": spack,
        }
        _CACHE["prep_shared"] = prep

    d = dict(prep)
    ix = np.zeros((128, IX_NCOL), np.int32)
    art = f(inputs["article_in"])[sl].astype(np.int32)   # [2, 2000]
    for b in range(BPC):
        for c in range(NTCH):
            pc = TCH[c]
            ix[:pc, IX_ART + b * NTCH + c] = art[b, c * 128:c * 128 + pc]
    q = f(inputs["question_in"])[sl].astype(np.int32)
    for b in range(BPC):
        ix[0:TQ, IX_Q + b] = q[b]
    for b in range(BPC):
        for o in range(4):
            ix[0:TO, IX_OPT + b * 4 + o] = \
                f(inputs[f"option{o + 1}_in"])[sl][b].astype(np.int32)
    d["idxpack"] = ix
    return d


def _get_runner():
    """jit-compiled 8-core runner, built once per process."""
    if "runner" in _CACHE:
        return _CACHE["runner"]
    import jax
    from jax.sharding import Mesh, PartitionSpec
    from jax.experimental.shard_map import shard_map
    from concourse.bass2jax import (_bass_exec_p, install_neuronx_cc_hook,
                                    partition_id_tensor)

    install_neuronx_cc_hook()
    nc = _get_nc()
    pid_name = nc.partition_id_tensor.name if nc.partition_id_tensor else None

    in_names, out_names, out_avals, zero_outs = [], [], [], []
    for alloc in nc.m.functions[0].allocations:
        if not isinstance(alloc, mybir.MemoryLocationSet):
            continue
        name = alloc.memorylocations[0].name
        if alloc.kind == "ExternalInput":
            if name != pid_name:
                in_names.append(name)
        elif alloc.kind == "ExternalOutput":
            out_names.append(name)
            shape = tuple(alloc.tensor_shape)
            dtype = mybir.dt.np(alloc.dtype)
            out_avals.append(jax.core.ShapedArray(shape, dtype))
            zero_outs.append(np.zeros(shape, dtype))
    n_params = len(in_names)
    all_in_names = in_names + out_names
    if pid_name is not None:
        all_in_names = all_in_names + [pid_name]

    def _body(*args):
        operands = list(args)
        if pid_name is not None:
            operands.append(partition_id_tensor())
        outs = _bass_exec_p.bind(
            *operands, out_avals=tuple(out_avals), in_names=tuple(all_in_names),
            out_names=tuple(out_names), lowering_input_output_aliases=(),
            sim_require_finite=True, sim_require_nnan=True, nc=nc)
        return tuple(outs)

    devices = jax.devices()[:NCORES]
    mesh = Mesh(np.asarray(devices), ("core",))
    in_specs = (PartitionSpec("core"),) * (n_params + len(out_names))
    out_specs = (PartitionSpec("core"),) * len(out_names)
    sharded = jax.jit(shard_map(_body, mesh=mesh, in_specs=in_specs,
                                out_specs=out_specs, check_rep=False),
                      keep_unused=True)

    _CACHE["runner"] = (sharded, in_names, out_names, zero_outs)
    return _CACHE["runner"]


def run_cores(per_core_inputs):
    """per_core_inputs: list of 8 dicts name->np array. Returns out dicts."""
    sharded, in_names, out_names, zero_outs = _get_runner()
    concat_in = [np.concatenate([per_core_inputs[c][n] for c in range(NCORES)],
                                axis=0) for n in in_names]
    concat_zeros = [np.concatenate([z] * NCORES, axis=0) for z in zero_outs]
    outs = sharded(*concat_in, *concat_zeros)
    result = []
    for c in range(NCORES):
        d = {}
        for i, n in enumerate(out_names):
            arr = np.asarray(outs[i])
            per = arr.shape[0] // NCORES
            d[n] = arr[c * per:(c + 1) * per]
        result.append(d)
    return result


def prepare_device_args(per_core_inputs):
    """device_put the concatenated inputs once, for repeated timed runs."""
    import jax
    from jax.sharding import Mesh, PartitionSpec, NamedSharding
    sharded, in_names, out_names, zero_outs = _get_runner()
    devices = jax.devices()[:NCORES]
    mesh = Mesh(np.asarray(devices), ("core",))
    sh = NamedSharding(mesh, PartitionSpec("core"))
    concat_in = [np.concatenate([per_core_inputs[c][n] for c in range(NCORES)],
                                axis=0) for n in in_names]
    concat_zeros = [np.concatenate([z] * NCORES, axis=0) for z in zero_outs]
    args = [jax.device_put(a, sh) for a in concat_in + concat_zeros]
    jax.block_until_ready(args)
    return args


def run_prepared(dev_args):
    sharded, in_names, out_names, zero_outs = _get_runner()
    outs = sharded(*dev_args)
    import jax
    jax.block_until_ready(outs)
    return outs


def kernel(**inputs):
    _CACHE.pop("prep_shared", None)
    per_core = [_prep_core_inputs(inputs, c) for c in range(NCORES)]
    res = run_cores(per_core)
    out = np.concatenate([res[c]["scores"] for c in range(NCORES)], axis=0)
    return out.astype(np.float32)


# revision 14
# speedup vs baseline: 1.1161x; 1.1161x over previous
"""BiAttentionMRU Trainium2 kernel (v2).

Data-parallel over batch: B=16 -> 2 batch elements on each of 8 cores.
All weights replicated. Embedding gather done on-device via indirect DMA.

v2 changes vs v1:
  - all constants packed into 3 DRAM tensors (idxpack / wmain / spack) so
    startup is 3 big DMAs instead of ~35 small serialized ones
  - index DMA + gathers emitted first so they overlap the weight loads
  - z/o/B1, CE and gate-mix drains batched over [100,1000] PSUM tiles
    (2 banks) -> half the ACT instructions
  - attention s1 accumulated in a [30,1000]x2 PSUM tile per b, single exp
    per half instead of 16 per-chunk exps
  - scal_dt derived on device by a cast (one less input)

Layouts (per core, per batch element b in {0,1}):
  art gathered as [t-chunk(128), d=300], PE-transposed into artT[dc][100, 2000]
  (d on partitions, 3 chunks of 100). Group sums, z/o/CE matmuls, gate mix,
  MRU scan (native tensor_tensor_scan along t) and the attention lhsT all
  work in [d, t] layout.

Attention algebra: aoq is never materialized. With e1 = exp(art_enc @ keys1^T),
Z1 its row sum, s2 = softmax-normalized p1 @ (q @ keys_f^T) is computed as
exp-of(u2 * 1/Z1) where u2 = e1 @ QK. The per-option mean over t of
softmax(s2) @ opt folds into one accumulating matmul sum_t e2[t,:] * (1/Z2[t]).
"""

import sys

sys.path.insert(0, "/opt/trn_rl_repo")

import numpy as np
import ml_dtypes

import concourse.bass as bass
import concourse.tile as tile
from concourse import bacc, mybir
from concourse.masks import make_identity

F32 = mybir.dt.float32
BF16 = mybir.dt.bfloat16
I32 = mybir.dt.int32
AX = mybir.AxisListType
OP = mybir.AluOpType
AF = mybir.ActivationFunctionType

DIM = 300
VOCAB = 50000
B_FULL = 16
NCORES = 8
BPC = B_FULL // NCORES  # batch per core = 2
T = 2000
TQ = 30
TO = 16
RANGES = (1, 2, 4, 10, 25)

TCH = [128] * 15 + [80]  # t chunking for transposes / attention
NTCH = len(TCH)
DC = 3  # d chunks of 100
DCS = 100

N_MM = 500   # matmul N-chunking (psum free <= 512 fp32)
PSW = 1024   # psum tile width (2 banks); matmul regions bank-aligned at 0/512

USE_BF16 = True
DT = BF16 if USE_BF16 else F32
NPDT = ml_dtypes.bfloat16 if USE_BF16 else np.float32

# ---- spack (f32 [128, SP_NCOL]) column layout ----
SP_BIAS = 0          # 30 cols: bias j of d-chunk kc at 10*kc + j
                     #   j: 0 bz, 1 bo, 2..6 ce_b[0..4], 7 f1_b, 8 f2_b, 9 f3_b
SP_SCAL = 30         # 24 cols: scalar table (see SC_* below, relative)
SP_AS1 = 54          # 6*75 cols: as1T [600,75] in 6 chunks of 100 rows
SP_AS2 = 504         # 1 col: as2T (75 rows)
SP_BAS1 = 505        # 1 col: as1_b (75 rows)
SP_NCOL = 506

# scalar table columns (relative to SP_SCAL)
SC_M1 = 0      # 15 cols: m1[k,r]/r at 5k+ri
SC_M1B = 15    # 3 cols
SC_M2 = 18     # 3 cols
SC_M2B = 21    # 1 col
SC_AS2B = 22   # 1 col
SC_NCOL = 24

# ---- wmain (bf16 [300, WM_NCOL] in DRAM -> [100, 3, WM_NCOL] sbuf) ----
WM_ART = 0       # 900: z (0:300) / o (300:600) / b1=ce0 (600:900)
WM_CE = 900      # 1200: ce r2/r4/r10/r25, 300 each
WM_F = 2100      # 900: f1 / f2 / f3
WM_NCOL = 3000

# ---- idxpack (i32 [128, IX_NCOL]) ----
IX_ART = 0    # 32 cols: b*16 + chunk (rows: t within chunk)
IX_Q = 32     # 2 cols: b (30 rows)
IX_OPT = 34   # 8 cols: b*4 + o (16 rows)
IX_NCOL = 42


def _build_program():
    nc = bacc.Bacc("TRN2", target_bir_lowering=False, debug=False,
                   num_devices=NCORES)

    emb = nc.dram_tensor("emb", [VOCAB, DIM], DT, kind="ExternalInput")
    idxpack = nc.dram_tensor("idxpack", [128, IX_NCOL], I32, kind="ExternalInput")
    wmain = nc.dram_tensor("wmain", [DC * DCS, WM_NCOL], DT, kind="ExternalInput")
    spack = nc.dram_tensor("spack", [128, SP_NCOL], F32, kind="ExternalInput")
    out = nc.dram_tensor("scores", [BPC, 4], F32, kind="ExternalOutput")

    with tile.TileContext(nc) as tc:
        from contextlib import ExitStack
        with ExitStack() as ctx:
            _emit(nc, tc, ctx, emb, idxpack, wmain, spack, out)

    nc.compile()
    return nc


def _emit(nc, tc, ctx, emb, idxpack, wmain, spack, out):
    # ---------------- pools ----------------
    consts = ctx.enter_context(tc.tile_pool(name="consts", bufs=1))
    pers = ctx.enter_context(tc.tile_pool(name="pers", bufs=1))
    gpool = ctx.enter_context(tc.tile_pool(name="gather", bufs=3))
    p_art = ctx.enter_context(tc.tile_pool(name="p_art", bufs=2))
    p_enc = ctx.enter_context(tc.tile_pool(name="p_enc", bufs=2))
    p_z = ctx.enter_context(tc.tile_pool(name="p_z", bufs=2))
    p_zb = ctx.enter_context(tc.tile_pool(name="p_zb", bufs=1))
    p_mix = ctx.enter_context(tc.tile_pool(name="p_mix", bufs=2))
    p_xs = ctx.enter_context(tc.tile_pool(name="p_xs", bufs=1))
    small = ctx.enter_context(tc.tile_pool(name="small", bufs=4))
    pp2 = ctx.enter_context(tc.tile_pool(name="pp2", bufs=2, space="PSUM"))
    ppwork = ctx.enter_context(tc.tile_pool(name="ppwork", bufs=2, space="PSUM"))
    ppacc = ctx.enter_context(tc.tile_pool(name="ppacc", bufs=2, space="PSUM"))

    # ---------------- index DMA + gathers first (overlap weight loads) ----
    ixs = pers.tile([128, IX_NCOL], I32, tag="ixs")
    nc.sync.dma_start(ixs[:], idxpack[:])

    gathered = []
    for b in range(BPC):
        g_art = []
        for c in range(NTCH):
            pc = TCH[c]
            g = gpool.tile([128, DIM], DT, tag="gart", name="gart")
            nc.gpsimd.indirect_dma_start(
                out=g[:pc, :], out_offset=None, in_=emb[:],
                in_offset=bass.IndirectOffsetOnAxis(
                    ap=ixs[:pc, IX_ART + b * NTCH + c:IX_ART + b * NTCH + c + 1],
                    axis=0))
            g_art.append(g)
        qg = pers.tile([TQ, DIM], DT, tag=f"qg{b}", name=f"qg{b}")
        nc.gpsimd.indirect_dma_start(
            out=qg[:], out_offset=None, in_=emb[:],
            in_offset=bass.IndirectOffsetOnAxis(
                ap=ixs[0:TQ, IX_Q + b:IX_Q + b + 1], axis=0))
        og = [pers.tile([TO, DIM], DT, tag=f"og{b}_{o}", name=f"og{b}_{o}")
              for o in range(4)]
        for o in range(4):
            nc.gpsimd.indirect_dma_start(
                out=og[o][:], out_offset=None, in_=emb[:],
                in_offset=bass.IndirectOffsetOnAxis(
                    ap=ixs[0:TO, IX_OPT + b * 4 + o:IX_OPT + b * 4 + o + 1],
                    axis=0))
        gathered.append(dict(g_art=g_art, qg=qg, og=og))

    # ---------------- constants / weights (3 big DMAs total) --------------
    ident = consts.tile([128, 128], DT)
    make_identity(nc, ident[:])

    wm = consts.tile([DCS, DC, WM_NCOL], DT)
    nc.sync.dma_start(wm[:], wmain[:].rearrange("(k p) j -> p k j", k=DC))
    sp = consts.tile([128, SP_NCOL], F32)
    nc.sync.dma_start(sp[:], spack[:])

    def sc(col):  # f32 per-partition scalar [100,1] from scal table
        return sp[0:DCS, SP_SCAL + col:SP_SCAL + col + 1]

    def bias(kc, j):  # f32 [100,1]
        return sp[0:DCS, SP_BIAS + 10 * kc + j:SP_BIAS + 10 * kc + j + 1]

    # scaled 100x100 identities for the PE-side gate mix:
    # cols j=5k+ri hold m1[k,ri]/r * I, cols 15+k hold m2[k] * I
    ones30 = consts.tile([TQ, 1], DT)
    nc.vector.memset(ones30[:], 1.0)
    mI = consts.tile([DCS, 18, DCS], DT)
    for j in range(18):
        scol = (SC_M1 + j) if j < 15 else (SC_M2 + j - 15)
        nc.vector.tensor_scalar_mul(mI[:, j, :], ident[0:DCS, 0:DCS], sc(scol))

    ans_sb = pers.tile([DCS, BPC, 6, 4], F32, tag="ans_sb")

    # ---------------- transposes (PE) -------------------------------------
    for b in range(BPC):
        g_art = gathered[b]["g_art"]
        artT = [p_art.tile([DCS, T], DT, tag=f"artT{dc}", name=f"artT{b}_{dc}")
                for dc in range(DC)]
        for c in range(NTCH):
            pc = TCH[c]
            for dc in range(DC):
                tp = ppwork.tile([DCS, 128], DT, tag="work")
                nc.tensor.transpose(tp[:, :pc],
                                    g_art[c][:pc, dc * DCS:(dc + 1) * DCS],
                                    ident[:pc, :pc])
                nc.vector.tensor_copy(artT[dc][:, c * 128:c * 128 + pc],
                                      tp[:, :pc])

        qg = gathered[b]["qg"]
        qT = pers.tile([DCS, DC, TQ], DT, tag=f"qT{b}")
        for dc in range(DC):
            tp = ppwork.tile([DCS, 128], DT, tag="work")
            nc.tensor.transpose(tp[:, :TQ], qg[:, dc * DCS:(dc + 1) * DCS],
                                ident[:TQ, :TQ])
            nc.vector.tensor_copy(qT[:, dc, :], tp[:, :TQ])

        og = gathered[b]["og"]
        oT = pers.tile([DCS, DC, 4, TO], DT, tag=f"oT{b}")
        for o in range(4):
            for dc in range(DC):
                tp = ppwork.tile([DCS, 128], DT, tag="work")
                nc.tensor.transpose(tp[:, :TO], og[o][:, dc * DCS:(dc + 1) * DCS],
                                    ident[:TO, :TO])
                nc.vector.tensor_copy(oT[:, dc, o, :], tp[:, :TO])
        gathered[b].update(artT=artT, qT=qT, oT=oT)

    # ---------------- per-b compute ---------------------------------------
    for b in range(BPC):
        artT = gathered[b]["artT"]
        qg = gathered[b]["qg"]
        qT = gathered[b]["qT"]
        og = gathered[b]["og"]
        oT = gathered[b]["oT"]

        # ---------------- group sums (xs_r in [d, g] layout) ----------------
        # xs2/xs4/xs10 on PE: accumulating identity matmuls over strided rhs
        # views (keeps GpSimd free for the gathers); xs25 via a DVE reduce.
        xs2 = [p_xs.tile([DCS, T // 2], DT, tag=f"xs2_{dc}", name=f"xs2_{dc}") for dc in range(DC)]
        xs4 = [p_xs.tile([DCS, T // 4], DT, tag=f"xs4_{dc}", name=f"xs4_{dc}") for dc in range(DC)]
        xs10 = [p_xs.tile([DCS, T // 10], DT, tag=f"xs10_{dc}", name=f"xs10_{dc}") for dc in range(DC)]
        xs25 = [p_xs.tile([DCS, T // 25], DT, tag=f"xs25_{dc}", name=f"xs25_{dc}") for dc in range(DC)]
        idq = ident[0:DCS, 0:DCS]
        for dc in range(DC):
            a = artT[dc]
            a3 = a[:].rearrange("p (g r) -> p g r", r=2)       # [100, 1000, 2]
            ps = pp2.tile([DCS, PSW], F32, tag="mm")
            for g0, gn in ((0, 512), (512, 488)):
                for j in range(2):
                    nc.tensor.matmul(ps[:, g0:g0 + gn], idq,
                                     a3[:, g0:g0 + gn, j],
                                     start=(j == 0), stop=(j == 1))
            nc.vector.tensor_copy(xs2[dc][:], ps[:, 0:T // 2])
            x23 = xs2[dc][:].rearrange("p (g r) -> p g r", r=2)  # [100, 500, 2]
            x25 = xs2[dc][:].rearrange("p (g r) -> p g r", r=5)  # [100, 200, 5]
            ps = pp2.tile([DCS, PSW], F32, tag="mm")
            for j in range(2):
                nc.tensor.matmul(ps[:, 0:T // 4], idq, x23[:, :, j],
                                 start=(j == 0), stop=(j == 1))
            for j in range(5):
                nc.tensor.matmul(ps[:, 512:512 + T // 10], idq, x25[:, :, j],
                                 start=(j == 0), stop=(j == 4))
            nc.vector.tensor_copy(xs4[dc][:], ps[:, 0:T // 4])
            nc.vector.tensor_copy(xs10[dc][:], ps[:, 512:512 + T // 10])
            with nc.allow_low_precision(reason="bf16 group sums feed bf16 matmuls"):
                nc.vector.tensor_reduce(
                    xs25[dc][:], a[:].rearrange("p (g r) -> p g r", r=25),
                    AX.X, OP.add)

        # ---------------- z / o / B1 (art stream, batched drains) -----------
        z_sb = [p_z.tile([DCS, T], DT, tag=f"z{dc}", name=f"z{dc}") for dc in range(DC)]
        o_sb = [p_enc.tile([DCS, T], DT, tag=f"o{dc}", name=f"o{dc}") for dc in range(DC)]
        b1_sb = [p_zb.tile([DCS, T], DT, tag=f"b1_{dc}", name=f"b1_{dc}") for dc in range(DC)]
        for mi, (dst, func, bcol) in enumerate(
                ((z_sb, AF.Tanh, 0), (o_sb, AF.Tanh, 1), (b1_sb, AF.Relu, 2))):
            for dc in range(DC):
                mcol = WM_ART + mi * DIM + dc * DCS
                for t0 in range(0, T, PSW):
                    w = min(PSW, T - t0)
                    ps = pp2.tile([DCS, PSW], F32, tag="mm")
                    for kc in range(DC):
                        for s0, sn in ((0, 512), (512, w - 512)):
                            nc.tensor.matmul(
                                ps[:, s0:s0 + sn],
                                wm[:, kc, mcol:mcol + DCS],
                                artT[kc][:, t0 + s0:t0 + s0 + sn],
                                start=(kc == 0), stop=(kc == DC - 1))
                    nc.scalar.activation(dst[dc][:, t0:t0 + w], ps[:, :w],
                                         func, bias=bias(dc, bcol))

        # ---------------- CE r>=2 -------------------------------------------
        bl = {}
        for ri, (xs, r) in enumerate(((xs2, 2), (xs4, 4), (xs10, 10), (xs25, 25))):
            g_r = T // r
            bl[r] = [p_xs.tile([DCS, g_r], DT, tag=f"bl{r}_{dc}", name=f"bl{r}_{dc}")
                     for dc in range(DC)]
            for dc in range(DC):
                mcol = WM_CE + ri * DIM + dc * DCS
                ps = pp2.tile([DCS, PSW], F32, tag="mm")
                regions = [(0, min(g_r, 512))]
                if g_r > 512:
                    regions.append((512, g_r - 512))
                for kc in range(DC):
                    for g0, gn in regions:
                        nc.tensor.matmul(
                            ps[:, g0:g0 + gn],
                            wm[:, kc, mcol:mcol + DCS],
                            xs[kc][:, g0:g0 + gn],
                            start=(kc == 0), stop=(kc == DC - 1))
                nc.scalar.activation(bl[r][dc][:, :], ps[:, :g_r],
                                     AF.Relu, bias=bias(dc, 3 + ri))

        # ---------------- gate mix ------------------------------------------
        # h1_k = relu(sum_r m1[k,r]/r * B_r^expand + m1_b[k]);
        # gate = relu(sum_k m2[k] h1_k + m2_b).
        # All three k and the gate combine on PE as scaled-identity
        # accumulating matmuls, bias folded into the ACT relu.
        # Expansion = stride-0 rhs views.
        gate = []
        for dc in range(DC):
            def ev_chunk(ri, t0, tn):
                r = RANGES[ri]
                if r == 1:
                    return b1_sb[dc][:, t0:t0 + tn]
                return bl[r][dc][:, t0 // r:(t0 + tn) // r, None] \
                    .to_broadcast([DCS, tn // r, r])

            h1 = []
            # psum regions bank-aligned at 0/512, each 500 wide; drained
            # through a [p, 2, 500] strided view in one ACT.
            def ps_view(ps):
                return ps[:].rearrange("p (g x) -> p g x", g=2)[:, :, 0:N_MM]

            def acc_view(acc, t0):
                return acc[:, t0:t0 + 2 * N_MM].rearrange(
                    "p (g x) -> p g x", g=2)

            for k in (0, 1, 2):
                acc = p_mix.tile([DCS, T], DT, tag=f"h1_{k}", name=f"h1_{k}")
                for t0 in range(0, T, 2 * N_MM):
                    ps = pp2.tile([DCS, PSW], F32, tag="mm")
                    for ri in range(5):
                        for reg, s0 in ((0, 0), (512, N_MM)):
                            nc.tensor.matmul(
                                ps[:, reg:reg + N_MM], mI[:, 5 * k + ri, :],
                                ev_chunk(ri, t0 + s0, N_MM),
                                start=(ri == 0), stop=(ri == 4))
                    nc.scalar.activation(acc_view(acc, t0), ps_view(ps),
                                         AF.Relu, bias=sc(SC_M1B + k))
                h1.append(acc)
            # gate combine on PE
            g_acc = p_mix.tile([DCS, T], DT, tag="gate")
            for t0 in range(0, T, 2 * N_MM):
                ps = pp2.tile([DCS, PSW], F32, tag="mm")
                for k in range(3):
                    for reg, s0 in ((0, 0), (512, N_MM)):
                        nc.tensor.matmul(
                            ps[:, reg:reg + N_MM], mI[:, 15 + k, :],
                            h1[k][:, t0 + s0:t0 + s0 + N_MM],
                            start=(k == 0), stop=(k == 2))
                nc.scalar.activation(acc_view(g_acc, t0), ps_view(ps),
                                     AF.Relu, bias=sc(SC_M2B))
            gate.append(g_acc)

        # ---------------- MRU scan + encode ---------------------------------
        encT = []
        for dc in range(DC):
            gz = p_mix.tile([DCS, T], DT, tag="gz", name="gz")
            nc.vector.tensor_mul(gz[:], gate[dc][:], z_sb[dc][:])
            nc.vector.tensor_sub(z_sb[dc][:], z_sb[dc][:], gz[:])  # (1-g)z
            c_t = p_mix.tile([DCS, T], DT, tag="c", name="c_t")
            nc.vector.tensor_tensor_scan(
                c_t[:], gate[dc][:], z_sb[dc][:], 0.0, op0=OP.mult, op1=OP.add)
            nc.vector.tensor_mul(o_sb[dc][:], o_sb[dc][:], c_t[:])
            encT.append(o_sb[dc])

        # ---------------- keys1T --------------------------------------------
        k1T = small.tile([DCS, DC, TQ], DT, tag="k1T")
        for dc in range(DC):
            ps = ppwork.tile([DCS, 128], F32, tag="work")
            for kc in range(DC):
                nc.tensor.matmul(ps[:, :TQ],
                                 wm[:, kc, WM_F + dc * DCS:WM_F + (dc + 1) * DCS],
                                 qT[:, kc, :], start=(kc == 0), stop=(kc == DC - 1))
            nc.scalar.copy(k1T[:, dc, :], ps[:, :TQ])

        # ---------------- A2/A3 and QK --------------------------------------
        aTs = []
        for fi in range(2):
            wcol = WM_F + (fi + 1) * DIM
            a_ps = ppwork.tile([TQ, DIM], F32, tag="work")
            for kc in range(DC):
                nc.tensor.matmul(a_ps[:], qT[:, kc, :],
                                 wm[:, kc, wcol:wcol + DIM],
                                 start=(kc == 0), stop=(kc == DC - 1))
            a_sb = small.tile([TQ, DIM], DT, tag="a_sb")
            nc.vector.tensor_copy(a_sb[:], a_ps[:])
            aT = small.tile([DCS, DC, TQ], DT, tag=f"aT{fi}")
            for dc in range(DC):
                tp = ppwork.tile([DCS, 128], DT, tag="work")
                nc.tensor.transpose(tp[:, :TQ], a_sb[:, dc * DCS:(dc + 1) * DCS],
                                    ident[:TQ, :TQ])
                nc.vector.tensor_copy(aT[:, dc, :], tp[:, :TQ])
            aTs.append(aT)

        qk_ps = ppacc.tile([TQ, 128], F32, tag="acc")
        for fi in range(2):
            for o in range(4):
                gcol = 16 * (4 * fi + o)
                for kc in range(DC):
                    nc.tensor.matmul(qk_ps[:, gcol:gcol + 16],
                                     aTs[fi][:, kc, :], oT[:, kc, o, :],
                                     start=(kc == 0), stop=(kc == DC - 1))
        qk_sb = small.tile([TQ, 128], DT, tag="qk_sb")
        nc.vector.tensor_copy(qk_sb[:], qk_ps[:])

        # ---------------- attention: s1 batched, then per-chunk stream ------
        # s1 computed transposed (M=30) so exp lands directly in e1T layout.
        e1T = p_art.tile([TQ, T], DT, tag="e1T")
        for t0 in range(0, T, PSW):
            w = min(PSW, T - t0)
            s1 = pp2.tile([TQ, PSW], F32, tag="mm")
            for dc in range(DC):
                for s0, sn in ((0, 512), (512, w - 512)):
                    nc.tensor.matmul(s1[:, s0:s0 + sn], k1T[:, dc, :],
                                     encT[dc][:, t0 + s0:t0 + s0 + sn],
                                     start=(dc == 0), stop=(dc == DC - 1))
            nc.scalar.activation(e1T[:, t0:t0 + w], s1[:, :w], AF.Exp)

        pb_ps = ppacc.tile([128, 8], F32, tag="acc")
        for c in range(NTCH):
            pc = TCH[c]
            z1ps = ppwork.tile([128, 8], F32, tag="work")
            nc.tensor.matmul(z1ps[:pc, 0:1], e1T[:, c * 128:c * 128 + pc],
                             ones30[:], start=True, stop=True)
            z1 = small.tile([128, 2], F32, tag="z1")
            nc.vector.reciprocal(z1[:pc, 1:2], z1ps[:pc, 0:1])
            u2 = ppwork.tile([128, 128], F32, tag="work")
            nc.tensor.matmul(u2[:pc, :], e1T[:, c * 128:c * 128 + pc], qk_sb[:],
                             start=True, stop=True)
            e2 = small.tile([128, 128], F32, tag="e2")
            nc.scalar.activation(e2[:pc, :], u2[:pc, :], AF.Exp,
                                 scale=z1[:pc, 1:2])
            z2 = small.tile([128, 16], F32, tag="z2")
            nc.vector.tensor_reduce(z2[:pc, 0:8],
                                    e2[:pc, :].rearrange("p (g w) -> p g w", w=16),
                                    AX.X, OP.add)
            nc.vector.reciprocal(z2[:pc, 8:16], z2[:pc, 0:8])
            nc.tensor.matmul(pb_ps[:, :], e2[:pc, :], z2[:pc, 8:16],
                             start=(c == 0), stop=(c == NTCH - 1))

        # ---------------- answer vectors ------------------------------------
        pb_sb = small.tile([128, 8], DT, tag="pb_sb")
        nc.vector.tensor_copy(pb_sb[:], pb_ps[:])
        ans_ps = ppacc.tile([DCS, 24], F32, tag="acc")
        for g in range(8):
            fi, o = g // 4, g % 4
            pb16 = small.tile([TO, 1], DT, tag="pb16")
            nc.sync.dma_start(pb16[:], pb_sb[16 * g:16 * g + 16, g:g + 1])
            for dc in range(DC):
                j = fi * 3 + dc
                nc.tensor.matmul(ans_ps[:, j * 4 + o:j * 4 + o + 1],
                                 og[o][:, dc * DCS:(dc + 1) * DCS], pb16[:],
                                 start=True, stop=True)
        # 1/T of the mean-over-t lands here (cheaper than scaling rz2 per chunk)
        nc.vector.tensor_scalar_mul(
            ans_sb[:, b, :, :].rearrange("p j o -> p (j o)"), ans_ps[:], 1.0 / T)

    # ---------------- final MLP (both batches together) ----------------
    h_ps = ppwork.tile([75, 8], F32, tag="work")
    for j in range(6):
        # rhs columns = (b, o) pairs for chunk j of the 600-dim ans vector
        rhs = ans_sb[:, :, j, :]
        nc.tensor.matmul(h_ps[:], sp[0:DCS, SP_AS1 + j * 75:SP_AS1 + (j + 1) * 75],
                         rhs, start=(j == 0), stop=(j == 5))
    h_sb = small.tile([75, 8], F32, tag="h_sb")
    nc.scalar.activation(h_sb[:], h_ps[:], AF.Relu,
                         bias=sp[0:75, SP_BAS1:SP_BAS1 + 1])
    s_ps = ppacc.tile([8, 1], F32, tag="acc")
    nc.tensor.matmul(s_ps[:], h_sb[:], sp[0:75, SP_AS2:SP_AS2 + 1],
                     start=True, stop=True)
    s_sb = small.tile([8, 1], F32, tag="s_sb")
    nc.scalar.activation(s_sb[:], s_ps[:], AF.Identity,
                         bias=sp[0:8, SP_SCAL + SC_AS2B:SP_SCAL + SC_AS2B + 1])
    nc.sync.dma_start(out[:].rearrange("b o -> (b o)")[:, None], s_sb[:])


# ---------------------------------------------------------------------------
# host side
# ---------------------------------------------------------------------------

_CACHE = {}


def _get_nc():
    if "nc" not in _CACHE:
        _CACHE["nc"] = _build_program()
    return _CACHE["nc"]


def _prep_core_inputs(inputs, core):
    b0 = core * BPC
    sl = slice(b0, b0 + BPC)
    f = np.asarray
    prep = _CACHE.get("prep_shared")
    if prep is None:
        # core-independent tensors, computed once per kernel() call set
        Wz, Wo = f(inputs["Wz"]), f(inputs["Wo"])
        ceW = f(inputs["ce_W"])
        wmain = np.zeros((DC * DCS, WM_NCOL), np.float32)
        wmain[:, WM_ART + 0 * DIM:WM_ART + 1 * DIM] = Wz.T
        wmain[:, WM_ART + 1 * DIM:WM_ART + 2 * DIM] = Wo.T
        wmain[:, WM_ART + 2 * DIM:WM_ART + 3 * DIM] = ceW[0].T
        for ri in range(4):
            wmain[:, WM_CE + ri * DIM:WM_CE + (ri + 1) * DIM] = ceW[ri + 1].T
        # s2 = aoq @ f2W @ opt^T, so f2/f3 go in UNtransposed
        # (f1 builds keys1^T = f1W @ q^T and does need the transpose)
        wmain[:, WM_F + 0 * DIM:WM_F + 1 * DIM] = f(inputs["f1_W"]).T
        wmain[:, WM_F + 1 * DIM:WM_F + 2 * DIM] = f(inputs["f2_W"])
        wmain[:, WM_F + 2 * DIM:WM_F + 3 * DIM] = f(inputs["f3_W"])

        spack = np.zeros((128, SP_NCOL), np.float32)
        bias_cols = np.stack(
            [f(inputs["bz"]), f(inputs["bo"]),
             *[f(inputs["ce_b"])[i] for i in range(5)],
             f(inputs["f1_b"]), f(inputs["f2_b"]), f(inputs["f3_b"])],
            axis=1)  # [300, 10]
        for kc in range(DC):
            spack[0:DCS, SP_BIAS + 10 * kc:SP_BIAS + 10 * (kc + 1)] = \
                bias_cols[kc * DCS:(kc + 1) * DCS]
        m1 = f(inputs["mr1_W"])
        for k in range(3):
            for ri, r in enumerate(RANGES):
                spack[:, SP_SCAL + SC_M1 + 5 * k + ri] = m1[k, ri] / r
        spack[:, SP_SCAL + SC_M1B:SP_SCAL + SC_M1B + 3] = f(inputs["mr1_b"])[None, :]
        spack[:, SP_SCAL + SC_M2:SP_SCAL + SC_M2 + 3] = f(inputs["mr2_W"])[0][None, :]
        spack[:, SP_SCAL + SC_M2B] = f(inputs["mr2_b"])[0]
        spack[:, SP_SCAL + SC_AS2B] = f(inputs["as2_b"])[0]
        as1T = f(inputs["as1_W"]).T  # [600, 75]
        for j in range(6):
            spack[0:DCS, SP_AS1 + j * 75:SP_AS1 + (j + 1) * 75] = \
                as1T[j * DCS:(j + 1) * DCS]
        spack[0:75, SP_AS2] = f(inputs["as2_W"])[0]
        spack[0:75, SP_BAS1] = f(inputs["as1_b"])

        prep = {
            "emb": f(inputs["emb"]).astype(NPDT),
            "wmain": wmain.astype(NPDT),
            "spack": spack,
        }
        _CACHE["prep_shared"] = prep

    d = dict(prep)
    ix = np.zeros((128, IX_NCOL), np.int32)
    art = f(inputs["article_in"])[sl].astype(np.int32)   # [2, 2000]
    for b in range(BPC):
        for c in range(NTCH):
            pc = TCH[c]
            ix[:pc, IX_ART + b * NTCH + c] = art[b, c * 128:c * 128 + pc]
    q = f(inputs["question_in"])[sl].astype(np.int32)
    for b in range(BPC):
        ix[0:TQ, IX_Q + b] = q[b]
    for b in range(BPC):
        for o in range(4):
            ix[0:TO, IX_OPT + b * 4 + o] = \
                f(inputs[f"option{o + 1}_in"])[sl][b].astype(np.int32)
    d["idxpack"] = ix
    return d


def _get_runner():
    """jit-compiled 8-core runner, built once per process."""
    if "runner" in _CACHE:
        return _CACHE["runner"]
    import jax
    from jax.sharding import Mesh, PartitionSpec
    from jax.experimental.shard_map import shard_map
    from concourse.bass2jax import (_bass_exec_p, install_neuronx_cc_hook,
                                    partition_id_tensor)

    install_neuronx_cc_hook()
    nc = _get_nc()
    pid_name = nc.partition_id_tensor.name if nc.partition_id_tensor else None

    in_names, out_names, out_avals, zero_outs = [], [], [], []
    for alloc in nc.m.functions[0].allocations:
        if not isinstance(alloc, mybir.MemoryLocationSet):
            continue
        name = alloc.memorylocations[0].name
        if alloc.kind == "ExternalInput":
            if name != pid_name:
                in_names.append(name)
        elif alloc.kind == "ExternalOutput":
            out_names.append(name)
            shape = tuple(alloc.tensor_shape)
            dtype = mybir.dt.np(alloc.dtype)
            out_avals.append(jax.core.ShapedArray(shape, dtype))
            zero_outs.append(np.zeros(shape, dtype))
    n_params = len(in_names)
    all_in_names = in_names + out_names
    if pid_name is not None:
        all_in_names = all_in_names + [pid_name]

    def _body(*args):
        operands = list(args)
        if pid_name is not None:
            operands.append(partition_id_tensor())
        outs = _bass_exec_p.bind(
            *operands, out_avals=tuple(out_avals), in_names=tuple(all_in_names),
            out_names=tuple(out_names), lowering_input_output_aliases=(),
            sim_require_finite=True, sim_require_nnan=True, nc=nc)
        return tuple(outs)

    devices = jax.devices()[:NCORES]
    mesh = Mesh(np.asarray(devices), ("core",))
    in_specs = (PartitionSpec("core"),) * (n_params + len(out_names))
    out_specs = (PartitionSpec("core"),) * len(out_names)
    sharded = jax.jit(shard_map(_body, mesh=mesh, in_specs=in_specs,
                                out_specs=out_specs, check_rep=False),
                      keep_unused=True)

    _CACHE["runner"] = (sharded, in_names, out_names, zero_outs)
    return _CACHE["runner"]


def run_cores(per_core_inputs):
    """per_core_inputs: list of 8 dicts name->np array. Returns out dicts."""
    sharded, in_names, out_names, zero_outs = _get_runner()
    concat_in = [np.concatenate([per_core_inputs[c][n] for c in range(NCORES)],
                                axis=0) for n in in_names]
    concat_zeros = [np.concatenate([z] * NCORES, axis=0) for z in zero_outs]
    outs = sharded(*concat_in, *concat_zeros)
    result = []
    for c in range(NCORES):
        d = {}
        for i, n in enumerate(out_names):
            arr = np.asarray(outs[i])
            per = arr.shape[0] // NCORES
            d[n] = arr[c * per:(c + 1) * per]
        result.append(d)
    return result


def prepare_device_args(per_core_inputs):
    """device_put the concatenated inputs once, for repeated timed runs."""
    import jax
    from jax.sharding import Mesh, PartitionSpec, NamedSharding
    sharded, in_names, out_names, zero_outs = _get_runner()
    devices = jax.devices()[:NCORES]
    mesh = Mesh(np.asarray(devices), ("core",))
    sh = NamedSharding(mesh, PartitionSpec("core"))
    concat_in = [np.concatenate([per_core_inputs[c][n] for c in range(NCORES)],
                                axis=0) for n in in_names]
    concat_zeros = [np.concatenate([z] * NCORES, axis=0) for z in zero_outs]
    args = [jax.device_put(a, sh) for a in concat_in + concat_zeros]
    jax.block_until_ready(args)
    return args


def run_prepared(dev_args):
    sharded, in_names, out_names, zero_outs = _get_runner()
    outs = sharded(*dev_args)
    import jax
    jax.block_until_ready(outs)
    return outs


def kernel(**inputs):
    _CACHE.pop("prep_shared", None)
    per_core = [_prep_core_inputs(inputs, c) for c in range(NCORES)]
    res = run_cores(per_core)
    out = np.concatenate([res[c]["scores"] for c in range(NCORES)], axis=0)
    return out.astype(np.float32)
